# revision 9
# baseline (speedup 1.0000x reference)
"""Trainium2 Bass kernel for nn_KMeansClassifier (conv encoder + soft k-means).

Strategy:
  - Data-parallel conv encoder: batch 256 sharded 32 images/core across 8 cores.
    Convs are f32r (tf32-like) matmuls: conv1 via host-side im2col (contract 27),
    conv2/conv3 via 9 shifted matmuls over zero-padded SBUF tiles (contract 128).
    BN is folded into weights/bias on host; LeakyReLU via ACT Prelu(alpha=0.1).
  - Each core L2-normalizes its 32 embeddings, transposes them on the PE, and
    contributes [4096, 32] to a single AllGather.
  - Soft k-means runs replicated on every core in Gram space: G = X @ X.T
    [256,256] is built once; each iteration is dist = G @ r_colnorm, so the
    iteration loop never touches the 4096-dim feature space. The per-cluster
    mass (softmax denominator of the mu update) is folded into the next
    iteration's exp scale.
  - Output r [256,16] is identical on every core; the host returns core 0's.
"""
import sys

sys.path.insert(0, "/opt/trn_rl_repo")

import numpy as np

import concourse.bacc as bacc
import concourse.mybir as mybir
import concourse.tile as tile
from concourse.masks import make_identity
from concourse.bass_utils import run_bass_kernel_spmd

dt = mybir.dt
AF = mybir.ActivationFunctionType
ALU = mybir.AluOpType
AX = mybir.AxisListType

N_CORES = 8
NLOC = 32            # images per core
K = 16
FEAT = 4096
BN_EPS = 1e-3
SLOPE = 0.1
CT = 30.0

_TRACE = False
_DEBUG = False
LAST_EXEC_NS = None
_BUILD_CACHE = {}


def _build(n_upd):
    """Trace + compile the SPMD kernel for n_upd mu-updates (= num_iter + 1)."""
    nc = bacc.Bacc(trn_type="TRN2", target_bir_lowering=False, debug=False,
                   num_devices=N_CORES)

    patches = nc.dram_tensor("patches", [NLOC, 32, 1024], dt.float32,
                             kind="ExternalInput").ap()
    w1 = nc.dram_tensor("w1", [128, 128], dt.float32, kind="ExternalInput").ap()
    w2 = nc.dram_tensor("w2", [128, 9 * 256], dt.float32, kind="ExternalInput").ap()
    w3 = nc.dram_tensor("w3", [128, 9 * 128], dt.float32, kind="ExternalInput").ap()
    b1 = nc.dram_tensor("b1", [128, 1], dt.float32, kind="ExternalInput").ap()
    b2 = nc.dram_tensor("b2", [128, 2], dt.float32, kind="ExternalInput").ap()
    b3 = nc.dram_tensor("b3", [64, 1], dt.float32, kind="ExternalInput").ap()
    mu0t = nc.dram_tensor("mu0t", [FEAT, K], dt.float32, kind="ExternalInput").ap()
    zeros = nc.dram_tensor("zeros", [128, 2312], dt.float32,
                           kind="ExternalInput").ap()
    r_out = nc.dram_tensor("r_out", [N_CORES * NLOC, K], dt.float32,
                           kind="ExternalOutput").ap()
    dbg_emb = nc.dram_tensor("dbg_emb", [NLOC, FEAT], dt.float32,
                             kind="ExternalOutput").ap() if _DEBUG else None
    dbg_g = nc.dram_tensor("dbg_g", [128, 256], dt.float32,
                           kind="ExternalOutput").ap() if _DEBUG else None
    dbg_e = nc.dram_tensor("dbg_e", [16, 256], dt.float32,
                           kind="ExternalOutput").ap() if _DEBUG else None

    f32 = dt.float32
    f32r = dt.float32r

    with tile.TileContext(nc) as tc:
        with tc.tile_pool(name="static", bufs=1) as st, \
             tc.tile_pool(name="iterp", bufs=2) as itp, \
             tc.tile_pool(name="dram", bufs=1, space="DRAM") as dp:

            # ---------------- static SBUF state ----------------
            w1s = st.tile([128, 128], f32r)
            w2s = st.tile([128, 9 * 256], f32r)
            w3s = st.tile([128, 9 * 128], f32r)
            b1s = st.tile([128, 1], f32)
            b2s = st.tile([128, 2], f32)
            b3s = st.tile([64, 1], f32)
            mu0s = st.tile([128, 32 * K], f32r)
            ident = st.tile([32, 32], f32)
            ones128 = st.tile([128, 1], f32)
            g0 = st.tile([128, 256], f32)
            g1 = st.tile([128, 256], f32)
            data_local = st.tile([NLOC, FEAT], f32)
            stt = st.tile([NLOC, FEAT], f32)
            dtl = st.tile([128, 32 * NLOC], f32)
            dtf = st.tile([128, 32 * 256], f32r)
            # h1pad: one tile per image pair (2 imgs, 34x34 padded), reused
            # across groups; h2pad: 2 ktile-halves x 4 imgs 18x18 padded,
            # double buffered across groups. Zeroed once; ACT rewrites only
            # the interiors, borders stay zero.
            h1pad = [st.tile([128, 2 * 1156], f32r, name=f"h1pad{i}",
                             tag=f"h1pad{i}")
                     for i in range(2)]
            h2pad = [[st.tile([128, 4 * 324], f32r, name=f"h2pad{i}_{kt}",
                              tag=f"h2pad{i}_{kt}")
                      for kt in range(2)]
                     for i in range(2)]  # [buf][ktile]

            nc.sync.dma_start(w1s[:], w1.bitcast(f32r))
            nc.sync.dma_start(w2s[:], w2.bitcast(f32r))
            nc.sync.dma_start(w3s[:], w3.bitcast(f32r))
            nc.sync.dma_start(b1s[:], b1)
            nc.sync.dma_start(b2s[:], b2)
            nc.sync.dma_start(b3s[:], b3)
            nc.sync.dma_start(
                mu0s[:].rearrange("p (j k) -> p j k", j=32),
                mu0t.bitcast(f32r).rearrange("(j p) k -> p j k", j=32))
            make_identity(nc, ident[:])
            nc.vector.memset(ones128[:], 1.0)
            # f32r tiles can't be memset directly (ISA check); zero them by
            # DMA from a host-provided zeros buffer (DMA->f32r is legal).
            zr = zeros.bitcast(f32r)
            for t in h1pad:
                nc.sync.dma_start(t[:], zr)
            for bufs in h2pad:
                for t in bufs:
                    nc.sync.dma_start(t[:], zr[:, 0:1296])

            cc_in = dp.tile([FEAT, NLOC], f32)
            cc_out = dp.tile([N_CORES * FEAT, NLOC], f32)

            # ---------------- conv encoder ----------------
            with tc.tile_pool(name="pc13", bufs=5, space="PSUM") as pc13, \
                 tc.tile_pool(name="pc2", bufs=3, space="PSUM") as pc2, \
                 tc.tile_pool(name="convs", bufs=2) as cvp:

                for g in range(8):          # 8 groups of 4 images
                    pstack = cvp.tile([128, 1024], f32r, tag="pstack")
                    nc.sync.dma_start(
                        pstack[:], patches[4 * g:4 * g + 4].bitcast(f32r))

                    h2 = h2pad[g % 2]
                    h2v = [h2[kt][:].rearrange("p (j h w) -> p j h w",
                                               j=4, h=18)
                           for kt in range(2)]

                    for pr in range(2):      # image pairs within the group
                        h1 = h1pad[pr]
                        h1v = h1[:].rearrange("p (a h w) -> p a h w",
                                              a=2, h=34)
                        for a in range(2):   # conv1 per image (PE row-packed)
                            i = 2 * pr + a
                            for half in range(2):
                                ps = pc13.tile([128, 512], f32, tag="c13")
                                nc.tensor.matmul(
                                    ps[:], w1s[32 * i:32 * i + 32, :],
                                    pstack[32 * i:32 * i + 32,
                                           512 * half:512 * half + 512],
                                    start=True, stop=True,
                                    tile_position=(32 * i, 0))
                                nc.scalar.activation(
                                    h1v[:, a, 1 + 16 * half:17 + 16 * half,
                                        1:33],
                                    ps[:], AF.Prelu, bias=b1s[:], alpha=SLOPE)

                        for kt in range(2):  # conv2: 256 outC in two halves
                            ps2 = pc2.tile([128, 512], f32, tag="c2")
                            for pos in range(9):
                                r, s = divmod(pos, 3)
                                nc.tensor.matmul(
                                    ps2[:],
                                    w2s[:, pos * 256 + kt * 128:
                                        pos * 256 + kt * 128 + 128],
                                    h1v[:, :, r:r + 32:2, s:s + 32:2],
                                    start=(pos == 0), stop=(pos == 8))
                            for a in range(2):
                                j = 2 * pr + a
                                nc.scalar.activation(
                                    h2v[kt][:, j, 1:17, 1:17],
                                    ps2[:, 256 * a:256 * a + 256],
                                    AF.Prelu, bias=b2s[:, kt:kt + 1],
                                    alpha=SLOPE)

                    ps3 = pc13.tile([64, 256], f32, tag="c13")
                    n_mm = 0
                    for pos in range(9):     # conv3 over the 4-image group
                        r, s = divmod(pos, 3)
                        for ch in range(2):
                            nc.tensor.matmul(
                                ps3[:],
                                w3s[:, (pos * 2 + ch) * 64:
                                    (pos * 2 + ch) * 64 + 64],
                                h2v[ch][:, :, r:r + 16:2, s:s + 16:2],
                                start=(n_mm == 0), stop=(n_mm == 17))
                            n_mm += 1
                    c3o = cvp.tile([64, 256], f32, tag="c3o")
                    nc.scalar.activation(c3o[:], ps3[:], AF.Prelu,
                                         bias=b3s[:], alpha=SLOPE)
                    for j in range(4):       # embed rows: f = c*64 + (y*8+x)
                        n = 4 * g + j
                        nc.sync.dma_start(
                            data_local[n:n + 1, :].rearrange(
                                "p (c q) -> p c q", c=64),
                            c3o[:, 64 * j:64 * j + 64])

            # ---------------- normalize + local transpose ----------------
            nrm2 = st.tile([NLOC, 1], f32)
            inv2 = st.tile([NLOC, 1], f32)
            rstd = st.tile([NLOC, 1], f32)
            nc.vector.scalar_tensor_tensor(
                stt[:], data_local[:], 1.0, data_local[:],
                op0=ALU.mult, op1=ALU.mult, accum_out=nrm2[:])
            nc.vector.reciprocal(inv2[:], nrm2[:])
            nc.scalar.activation(rstd[:], inv2[:], AF.Sqrt)
            nc.vector.tensor_scalar_mul(data_local[:], data_local[:], rstd[:])

            if _DEBUG:
                nc.sync.dma_start(dbg_emb, data_local[:])
            with tc.tile_pool(name="pt", bufs=4, space="PSUM") as pt:
                for j in range(32):
                    ps = pt.tile([128, 32], f32, tag="tp")
                    nc.tensor.transpose(
                        ps[:], data_local[:, 128 * j:128 * j + 128], ident[:])
                    nc.vector.tensor_copy(dtl[:, 32 * j:32 * j + 32], ps[:])

            # ---------------- allgather ----------------
            nc.sync.dma_start(
                cc_in[:].rearrange("(j p) i -> p j i", j=32),
                dtl[:].rearrange("p (j i) -> p j i", j=32))
            nc.gpsimd.collective_compute(
                "AllGather", ALU.bypass,
                replica_groups=[list(range(N_CORES))],
                ins=[cc_in.opt()], outs=[cc_out.opt()])
            cov = cc_out[:].rearrange("(r f) i -> f r i", r=N_CORES)
            for j in range(32):
                nc.sync.dma_start(
                    dtf[:, 256 * j:256 * j + 256],
                    cov[128 * j:128 * (j + 1)].bitcast(f32r))

            # ---------------- gram matrix + kmeans ----------------
            with tc.tile_pool(name="pk", bufs=2, space="PSUM") as pk, \
                 tc.tile_pool(name="pkb", bufs=3, space="PSUM") as pkb, \
                 tc.tile_pool(name="pks", bufs=2, space="PSUM") as pks:

                for m, gm in enumerate((g0, g1)):
                    psg = pkb.tile([128, 256], f32, tag="big")
                    for j in range(32):
                        nc.tensor.matmul(
                            psg[:],
                            dtf[:, 256 * j + 128 * m:256 * j + 128 * m + 128],
                            dtf[:, 256 * j:256 * j + 256],
                            start=(j == 0), stop=(j == 31))
                    nc.vector.tensor_copy(gm[:], psg[:])
                if _DEBUG:
                    nc.sync.dma_start(dbg_g, g0[:])

                sc30 = None
                dt_ps = None
                for t in range(n_upd + 1):
                    rn = []
                    if t == 0:
                        # D0 = X @ mu0.T in [n,k] layout: mu0 is unnormalized,
                        # so dist can be O(30) -- subtract a per-row max
                        # before exp (folded into the ACT bias).
                        for h in range(2):
                            psd = pkb.tile([128, K], f32, tag="big")
                            for j in range(32):
                                nc.tensor.matmul(
                                    psd[:],
                                    dtf[:, 256 * j + 128 * h:
                                        256 * j + 128 * h + 128],
                                    mu0s[:, K * j:K * j + K],
                                    start=(j == 0), stop=(j == 31))
                            mx = itp.tile([128, 1], f32, tag="mx")
                            nc.vector.reduce_max(mx[:], psd[:], axis=AX.X)
                            negb = itp.tile([128, 1], f32, tag="negb")
                            nc.vector.tensor_scalar_mul(mx[:], mx[:], CT)
                            nc.vector.tensor_scalar_mul(negb[:], mx[:], -1.0)
                            e_nk = itp.tile([128, K], f32, tag="enk")
                            nc.scalar.activation(e_nk[:], psd[:], AF.Exp,
                                                 scale=CT, bias=negb[:])
                            s_h = itp.tile([128, 1], f32, tag="s")
                            nc.vector.reduce_sum(s_h[:], e_nk[:], axis=AX.X)
                            invs = itp.tile([128, 1], f32, tag="invs")
                            nc.vector.reciprocal(invs[:], s_h[:])
                            rn_h = itp.tile([128, K], f32, tag="rn")
                            nc.vector.tensor_scalar_mul(rn_h[:], e_nk[:],
                                                        invs[:])
                            rn.append(rn_h)
                    else:
                        et = itp.tile([16, 256], f32, tag="E")
                        nc.scalar.activation(et[:], dt_ps[:], AF.Exp,
                                             scale=sc30[:])
                        if _DEBUG and t == 1:
                            nc.sync.dma_start(dbg_e, et[:])
                        for h in range(2):
                            pse = pkb.tile([128, 16], f32, tag="big")
                            nc.tensor.transpose(
                                pse[:], et[:, 128 * h:128 * h + 128],
                                ident[0:16, 0:16])
                            s_h = itp.tile([128, 1], f32, tag="s")
                            nc.vector.reduce_sum(s_h[:], pse[:], axis=AX.X)
                            invs = itp.tile([128, 1], f32, tag="invs")
                            nc.vector.reciprocal(invs[:], s_h[:])
                            rn_h = itp.tile([128, 16], f32, tag="rn")
                            nc.vector.tensor_scalar_mul(rn_h[:], pse[:],
                                                        invs[:])
                            rn.append(rn_h)

                    if t < n_upd:
                        psden = pks.tile([1, 16], f32, tag="sm")
                        nc.tensor.matmul(psden[:], ones128[:], rn[0][:],
                                         start=True, stop=False)
                        nc.tensor.matmul(psden[:], ones128[:], rn[1][:],
                                         start=False, stop=True)
                        denS = itp.tile([1, 16], f32, tag="denS")
                        nc.vector.tensor_copy(denS[:], psden[:])
                        # [1,16] -> [16,1] via a K=1 matmul with rhs=[1]
                        psdt = pks.tile([16, 1], f32, tag="sm")
                        nc.tensor.matmul(psdt[:], denS[:], ones128[0:1, 0:1],
                                         start=True, stop=True)
                        invden = itp.tile([16, 1], f32, tag="invden")
                        nc.vector.reciprocal(invden[:], psdt[:])
                        sc30 = itp.tile([16, 1], f32, tag="sc30")
                        nc.vector.tensor_scalar_mul(sc30[:], invden[:], CT)

                        dt_ps = pk.tile([16, 256], f32, tag="dt")
                        nc.tensor.matmul(dt_ps[:], rn[0][:], g0[:],
                                         start=True, stop=False)
                        nc.tensor.matmul(dt_ps[:], rn[1][:], g1[:],
                                         start=False, stop=True)
                    else:
                        for h in range(2):
                            nc.sync.dma_start(
                                r_out[128 * h:128 * h + 128, :], rn[h][:])

    nc.compile()
    return nc


def _host_prep(x, conv1_w, conv1_b, bn1_g, bn1_b, bn1_m, bn1_v,
               conv2_w, conv2_b, bn2_g, bn2_b, bn2_m, bn2_v,
               conv3_w, conv3_b, bn3_g, bn3_b, bn3_m, bn3_v, mu0):
    f = np.float32

    def fold(w, b, g, beta, m, v):
        w = np.asarray(w, f)
        b = np.asarray(b, f)
        sc = (np.asarray(g, f) / np.sqrt(np.asarray(v, f) + BN_EPS)).astype(f)
        return (w * sc[:, None, None, None]).astype(f), \
               (b * sc + np.asarray(beta, f) - np.asarray(m, f) * sc).astype(f)

    W1, B1 = fold(conv1_w, conv1_b, bn1_g, bn1_b, bn1_m, bn1_v)
    W2, B2 = fold(conv2_w, conv2_b, bn2_g, bn2_b, bn2_m, bn2_v)
    W3, B3 = fold(conv3_w, conv3_b, bn3_g, bn3_b, bn3_m, bn3_v)

    w1t = W1.transpose(1, 2, 3, 0).reshape(27, 128)          # (c,ky,kx) x k
    w1p = np.zeros((32, 128), f)
    w1p[:27] = w1t
    w1h = np.ascontiguousarray(np.tile(w1p, (4, 1)))         # [128, 128]

    w2h = np.ascontiguousarray(np.concatenate(
        [W2[:, :, r, s].T for r in range(3) for s in range(3)],
        axis=1)).astype(f)                                   # [128, 2304]
    w3h = np.ascontiguousarray(np.concatenate(
        [W3[:, 128 * ch:128 * ch + 128, r, s].T
         for r in range(3) for s in range(3) for ch in range(2)],
        axis=1)).astype(f)                                   # [128, 1152]

    b1h = np.ascontiguousarray(B1.reshape(128, 1))
    b2h = np.ascontiguousarray(B2.reshape(2, 128).T)         # [:,kt] = B2[128kt:]
    b3h = np.ascontiguousarray(B3.reshape(64, 1))

    # conv1 im2col (stride 2, pad 1): patches[n, (c,ky,kx), (y,x)]
    xp = np.pad(np.asarray(x, f), ((0, 0), (0, 0), (1, 1), (1, 1)))
    s0, s1, s2, s3 = xp.strides
    pv = np.lib.stride_tricks.as_strided(
        xp, shape=(256, 3, 3, 3, 32, 32),
        strides=(s0, s1, s2, s3, 2 * s2, 2 * s3))
    pat = np.zeros((256, 32, 1024), f)
    pat[:, :27] = pv.reshape(256, 27, 1024)

    mu0t = np.ascontiguousarray(np.asarray(mu0, f).T)        # [4096, 16]
    return pat, w1h, w2h, w3h, b1h, b2h, b3h, mu0t


def kernel(x, conv1_w, conv1_b, bn1_g, bn1_b, bn1_m, bn1_v,
           conv2_w, conv2_b, bn2_g, bn2_b, bn2_m, bn2_v,
           conv3_w, conv3_b, bn3_g, bn3_b, bn3_m, bn3_v,
           mu0, num_iter):
    global LAST_EXEC_NS
    n_upd = int(np.asarray(num_iter)) + 1
    if n_upd not in _BUILD_CACHE:
        _BUILD_CACHE[n_upd] = _build(n_upd)
    nc = _BUILD_CACHE[n_upd]

    pat, w1h, w2h, w3h, b1h, b2h, b3h, mu0t = _host_prep(
        x, conv1_w, conv1_b, bn1_g, bn1_b, bn1_m, bn1_v,
        conv2_w, conv2_b, bn2_g, bn2_b, bn2_m, bn2_v,
        conv3_w, conv3_b, bn3_g, bn3_b, bn3_m, bn3_v, mu0)

    shared = {"w1": w1h, "w2": w2h, "w3": w3h, "b1": b1h, "b2": b2h,
              "b3": b3h, "mu0t": mu0t,
              "zeros": np.zeros((128, 2312), np.float32)}
    in_maps = [{"patches": np.ascontiguousarray(pat[NLOC * c:NLOC * (c + 1)]),
                **shared} for c in range(N_CORES)]

    res = run_bass_kernel_spmd(nc, in_maps, core_ids=list(range(N_CORES)),
                               trace=_TRACE)
    LAST_EXEC_NS = res.exec_time_ns
    return np.asarray(res.results[0]["r_out"])
